# revision 24
# baseline (speedup 1.0000x reference)
"""AttentionOnAttention Trainium2 kernel (8 NeuronCores, SPMD).

Sharding: core c handles batch b = c//4 and heads [4*(c%4), 4*(c%4)+4);
each core computes the disjoint output slice out[b, :, 256*(c%4):...] so no
collectives are needed.

Per-core dataflow (transposed orientation, partition = feature dim), bf16
matmuls with fp32 PSUM accumulation:
  xT --PE--> qTp/kTp (head-PAIRED [128, n]: even head on partitions 0:64,
             odd head on 64:128) and v (natural, ones-augmented)
  S^T pair = k_h q_h^T for both heads of a pair CONCURRENTLY via
             tile_position row packing (K=64 each, rows 0:64 + 64:128)
  expS^T   = exp(S^T / 8)  (ScalarE, scale fused; this is the kernel's
             roofline: ~128 x [128,1024] ACTIVATEs)
  [ao^T; L] = [v|1]^T expS^T  (ones column gives softmax denominators free)
  ao^T /= L  (DVE reciprocal + GPSIMD partition broadcast + DVE mul, lazy)
  [I^T; G^T] = Wcq^T qT + Wca^T ao^T   (output+gate linears fused, K=64)
  out^T = (I^T + b) * (0.5 + 0.5 tanh((G^T + bg)/2))  (sigmoid via tanh:
          same ACT table set as exp, so no table-set switch)

The projection of the second head pair, the v projections 8..15, and the
AoA linears are emitted as FILLER units inside the attention j-loops so they
run in TensorE slack while ScalarE (the bottleneck) streams exps.
"""

import numpy as np
from contextlib import ExitStack

import concourse.bass as bass
import concourse.bacc as bacc
import concourse.tile as tile
from concourse import mybir

B, N, DIM, H, DH = 2, 2048, 1024, 16, 64
HPC = H // 4          # 4 heads per core
INC = HPC * DH        # 256 per-core inner width
KT = DIM // 128       # 8 contraction tiles
NCH = N // 512        # 4 free-dim chunks of 512
JT = N // 128         # 16 j tiles
SCALE = float(DH) ** -0.5
F32 = mybir.dt.float32
BF16 = mybir.dt.bfloat16
AF = mybir.ActivationFunctionType
ALU = mybir.AluOpType


def build_nc():
    nc = bacc.Bacc(
        "TRN2",
        target_bir_lowering=False,
        debug=False,
        enable_asserts=False,
        num_devices=8,
    )
    xT_d = nc.dram_tensor("xT", (KT, 128, N), BF16, kind="ExternalInput").ap()
    wq_d = nc.dram_tensor("wq", (DIM, INC), BF16, kind="ExternalInput").ap()
    wk_d = nc.dram_tensor("wk", (DIM, INC), BF16, kind="ExternalInput").ap()
    wv_d = nc.dram_tensor("wv", (DIM, INC), BF16, kind="ExternalInput").ap()
    wcq_d = nc.dram_tensor("wcq", (DH, 2 * DH), BF16, kind="ExternalInput").ap()
    wca_d = nc.dram_tensor("wca", (DH, 2 * DH), BF16, kind="ExternalInput").ap()
    bias_d = nc.dram_tensor("biases", (2 * DH, 1), F32, kind="ExternalInput").ap()
    outT_d = nc.dram_tensor("outT", (INC, N), F32, kind="ExternalOutput").ap()

    with tile.TileContext(nc) as tc, ExitStack() as ctx:
        consts = ctx.enter_context(tc.tile_pool(name="consts", bufs=1))
        psum = ctx.enter_context(tc.tile_pool(name="psum", bufs=3, space="PSUM"))

        # AoA weights
        wcq_sb = consts.tile([DH, 2 * DH], BF16, name="wcq_sb")
        nc.sync.dma_start(out=wcq_sb, in_=wcq_d)
        wca_sb = consts.tile([DH, 2 * DH], BF16, name="wca_sb")
        nc.sync.dma_start(out=wca_sb, in_=wca_d)
        bias_sb = consts.tile([2 * DH, 1], F32, name="bias_sb")
        nc.sync.dma_start(out=bias_sb, in_=bias_d)

        # Prefetch the exp/tanh ACT table set during the DMA prologue so the
        # first attention exp doesn't stall PE long enough to re-throttle HAM.
        warm_sb = consts.tile([128, 1], F32, name="warm_sb")
        nc.scalar.activation(out=warm_sb, in_=bias_sb, func=AF.Exp)
        nc.scalar.activation(out=warm_sb, in_=warm_sb, func=AF.Tanh)

        # persistent tensors
        qTp = [consts.tile([128, N], BF16, name=f"qTp{p}") for p in range(2)]
        qTo = [consts.tile([DH, N], BF16, name=f"qTo{p}") for p in range(2)]
        kTp = [consts.tile([128, N], BF16, name=f"kTp{p}") for p in range(2)]
        v_aug = consts.tile([128, JT, HPC, DH + 1], BF16, name="v_aug")
        nc.vector.memset(v_aug[:, :, :, DH : DH + 1], 1.0)
        aoT = [consts.tile([DH, N], BF16, name=f"aoT{h}") for h in range(HPC)]
        aoU = [consts.tile([DH + 1, N], F32, name=f"aoU{h}") for h in range(HPC)]
        IG_sb = [consts.tile([128, N], F32, name=f"IG{h}") for h in range(HPC)]

        esp = ctx.enter_context(tc.tile_pool(name="es_p", bufs=4))
        nrm = ctx.enter_context(tc.tile_pool(name="norm_p", bufs=4))
        xw = tc.alloc_tile_pool(name="xw", bufs=1)
        # DMA order: first projection needs wq + wk + xt; wv afterwards.
        wq_sb = xw.tile([128, KT, INC], BF16, name="wq_sb")
        wk_sb = xw.tile([128, KT, INC], BF16, name="wk_sb")
        wv_sb = xw.tile([128, KT, INC], BF16, name="wv_sb")
        xt_sb = xw.tile([128, KT, N], BF16, name="xt_sb")
        for k in range(KT):
            ks = slice(k * 128, (k + 1) * 128)
            nc.sync.dma_start(out=wq_sb[:, k, :], in_=wq_d[ks, :])
            nc.sync.dma_start(out=wk_sb[:, k, :], in_=wk_d[ks, :])
        for k in range(KT):
            nc.sync.dma_start(out=xt_sb[:, k, 0:1024], in_=xT_d[k, :, 0:1024])
        for k in range(KT):
            ks = slice(k * 128, (k + 1) * 128)
            nc.sync.dma_start(out=wv_sb[:, k, :], in_=wv_d[ks, :])
        for k in range(KT):
            nc.sync.dma_start(out=xt_sb[:, k, 1024:N], in_=xT_d[k, :, 1024:N])

        # ---------------- projection units ----------------
        def qk_unit(wsb, dst, p, c):
            cs = slice(c * 512, (c + 1) * 512)
            ps = psum.tile([128, 1024], F32, name="ps_qk", tag="spair")
            for k in range(KT):
                nc.tensor.matmul(
                    ps[:, 0:512],
                    lhsT=wsb[:, k, p * 128 : (p + 1) * 128],
                    rhs=xt_sb[:, k, cs],
                    start=(k == 0),
                    stop=(k == KT - 1),
                )
            nc.vector.tensor_copy(out=dst[p][:, cs], in_=ps[:, 0:512])
            if dst is qTp:
                nc.vector.tensor_copy(out=qTo[p][:, cs], in_=ps[DH:128, 0:512])

        def v_unit(it):
            psv = psum.tile([128, 1024], F32, name="ps_v", tag="spair")
            for k in range(KT):
                nc.tensor.matmul(
                    psv[:, 0:INC],
                    lhsT=xt_sb[:, k, it * 128 : (it + 1) * 128],
                    rhs=wv_sb[:, k, :],
                    start=(k == 0),
                    stop=(k == KT - 1),
                )
            nc.vector.tensor_copy(
                out=v_aug[:, it, :, 0:DH],
                in_=psv[:, 0:INC].rearrange("p (h d) -> p h d", h=HPC),
            )

        def aoa_unit(h, half):
            p = h // 2
            odd = h % 2
            ig = psum.tile([128, 1024], F32, name="ig", tag="spair")
            for cc in range(2):
                c = 2 * half + cc
                cs = slice(c * 512, (c + 1) * 512)
                igs = ig[:, cc * 512 : (cc + 1) * 512]
                qrhs = qTo[p][:, cs] if odd else qTp[p][0:DH, cs]
                nc.tensor.matmul(
                    igs, lhsT=wcq_sb, rhs=qrhs, start=True, stop=False
                )
                nc.tensor.matmul(
                    igs, lhsT=wca_sb, rhs=aoT[h][:, cs], start=False, stop=True
                )
            nc.vector.tensor_copy(
                out=IG_sb[h][:, half * 1024 : (half + 1) * 1024], in_=ig
            )

        # ---------------- prefix: pair-0 q/k + v tiles 0..7 ----------------
        for c in range(NCH):
            qk_unit(wq_sb, qTp, 0, c)
            qk_unit(wk_sb, kTp, 0, c)
        for it in range(8):
            v_unit(it)

        # ---------------- attention (+ fillers) ----------------
        def norm_chain(h, c):
            """aoT[h][:, chunk c] = ao_unnorm / L, off the critical path."""
            cs = slice(c * 512, (c + 1) * 512)
            rl = nrm.tile([1, 512], F32, name="rl", tag="rl")
            nc.vector.reciprocal(out=rl, in_=aoU[h][DH : DH + 1, cs])
            rlb = nrm.tile([DH, 512], F32, name="rlb", tag="rlb")
            nc.gpsimd.partition_broadcast(rlb, rl)
            nc.vector.tensor_mul(out=aoT[h][:, cs], in0=aoU[h][0:DH, cs], in1=rlb)

        for p in range(2):
            # filler queue for this pair: (closure, pace_in_steps)
            fillers = []
            if p == 0:
                for it in range(8, JT):
                    fillers.append((lambda it=it: v_unit(it), 1))
                for c in range(NCH):
                    fillers.append((lambda c=c: qk_unit(wq_sb, qTp, 1, c), 2))
                    fillers.append((lambda c=c: qk_unit(wk_sb, kTp, 1, c), 2))
            else:
                for h in (0, 1):
                    for half in (0, 1):
                        fillers.append(
                            (lambda h=h, half=half: aoa_unit(h, half), 4)
                        )
            cooldown = 0

            for c in range(NCH):
                cs = slice(c * 512, (c + 1) * 512)
                pv = [
                    psum.tile([DH + 1, 512], F32, name=f"pv{hh}", tag="pv", bufs=2)
                    for hh in range(2)
                ]
                es_tiles = [None] * JT

                def emit_pv(jt):
                    for hh in range(2):
                        nc.tensor.matmul(
                            pv[hh],
                            lhsT=v_aug[:, jt, 2 * p + hh, :],
                            rhs=es_tiles[jt][:, hh * 512 : (hh + 1) * 512],
                            start=(jt == 0),
                            stop=(jt == JT - 1),
                        )

                for jt in range(JT):
                    jts = slice(jt * 128, (jt + 1) * 128)
                    s = psum.tile([128, 1024], F32, name="s", tag="spair")
                    nc.tensor.matmul(
                        s[:, 0:512],
                        lhsT=kTp[p][0:DH, jts],
                        rhs=qTp[p][0:DH, cs],
                        start=True,
                        stop=True,
                        tile_position=(0, 0),
                    )
                    nc.tensor.matmul(
                        s[:, 512:1024],
                        lhsT=kTp[p][DH:128, jts],
                        rhs=qTp[p][DH:128, cs],
                        start=True,
                        stop=True,
                        tile_position=(64, 0),
                    )
                    es = esp.tile([128, 1024], BF16, name="es", tag="es")
                    nc.scalar.activation(out=es, in_=s, func=AF.Exp, scale=SCALE)
                    es_tiles[jt] = es
                    # keep PE one S-tile ahead of the PV consumer
                    if jt > 0:
                        emit_pv(jt - 1)
                    if fillers and cooldown <= 0:
                        fn, pace = fillers.pop(0)
                        fn()
                        cooldown = pace
                    else:
                        cooldown -= 1
                emit_pv(JT - 1)

                # evacuate PSUM quickly, then normalize lazily on DVE/GPSIMD
                for hh in range(2):
                    h = 2 * p + hh
                    nc.vector.tensor_copy(out=aoU[h][:, cs], in_=pv[hh])
                    norm_chain(h, c)

            if p == 0:
                xw.release()

        # ---------------- AoA for the second pair + finals ----------------
        for h in (2, 3):
            for half in (0, 1):
                aoa_unit(h, half)

        fin = ctx.enter_context(tc.tile_pool(name="fin_p", bufs=2))
        for h in range(HPC):
            # sigmoid(G + bg) = 0.5 + 0.5*tanh((G + bg)/2); bias_sb[64:] = bg/2
            tg = fin.tile([DH, N], F32, name="tg", tag="tg")
            nc.scalar.activation(
                out=tg,
                in_=IG_sb[h][DH:128, :],
                func=AF.Tanh,
                scale=0.5,
                bias=bias_sb[DH : 2 * DH, :],
            )
            sg = fin.tile([DH, N], F32, name="sg", tag="sg")
            nc.vector.tensor_scalar(
                out=sg, in0=tg, scalar1=0.5, scalar2=0.5, op0=ALU.mult, op1=ALU.add
            )
            ot = fin.tile([DH, N], F32, name="ot", tag="ot")
            nc.vector.scalar_tensor_tensor(
                out=ot,
                in0=IG_sb[h][0:DH, :],
                scalar=bias_sb[0:DH, :],
                in1=sg,
                op0=ALU.add,
                op1=ALU.mult,
            )
            for half in range(2):
                hs = slice(half * 1024, (half + 1) * 1024)
                nc.sync.dma_start(out=outT_d[h * DH : (h + 1) * DH, hs], in_=ot[:, hs])

    nc.compile()
    return nc


_NC_CACHE = None


def _get_nc():
    global _NC_CACHE
    if _NC_CACHE is None:
        _NC_CACHE = build_nc()
    return _NC_CACHE


def make_in_maps(x, Wq, Wkv, Wq_out, Wattn_out, out_bias, Wq_gate, Wattn_gate,
                 gate_bias):
    import ml_dtypes

    bf16 = ml_dtypes.bfloat16
    wcq = np.ascontiguousarray(np.concatenate([Wq_out.T, Wq_gate.T], axis=1),
                               dtype=bf16)
    wca = np.ascontiguousarray(
        np.concatenate([Wattn_out.T, Wattn_gate.T], axis=1), dtype=bf16
    )
    biases = np.concatenate(
        [out_bias.reshape(-1), 0.5 * gate_bias.reshape(-1)]
    ).astype(np.float32).reshape(2 * DH, 1)
    biases = np.ascontiguousarray(biases)
    Wk = Wkv[:, : H * DH]
    Wv = Wkv[:, H * DH :]
    xT = [
        np.ascontiguousarray(x[b].T.reshape(KT, 128, N)).astype(bf16)
        for b in range(B)
    ]
    in_maps = []
    for c in range(8):
        b, hg = c // 4, c % 4
        cols = slice(hg * INC, (hg + 1) * INC)
        in_maps.append(
            {
                "xT": xT[b],
                "wq": np.ascontiguousarray(Wq[:, cols]).astype(bf16),
                "wk": np.ascontiguousarray(Wk[:, cols]).astype(bf16),
                "wv": np.ascontiguousarray(Wv[:, cols]).astype(bf16),
                "wcq": wcq,
                "wca": wca,
                "biases": biases,
            }
        )
    return in_maps


def assemble_output(results):
    out = np.empty((B, N, H * DH), dtype=np.float32)
    for c in range(8):
        b, hg = c // 4, c % 4
        out[b, :, hg * INC : (hg + 1) * INC] = results[c]["outT"].T
    return out


def kernel(**inputs):
    from concourse.bass_utils import run_bass_kernel_spmd

    inputs = {k: np.asarray(v, dtype=np.float32) for k, v in inputs.items()}
    nc = _get_nc()
    in_maps = make_in_maps(**inputs)
    res = run_bass_kernel_spmd(nc, in_maps, core_ids=list(range(8)))
    return assemble_output(res.results)
